# revision 36
# baseline (speedup 1.0000x reference)
"""AttnBlock (GroupNorm + 1x1-conv QKV + single-head spatial attention + proj
+ residual) on 8 Trainium2 NeuronCores.

Sharding: pure data-parallel over batch — 16 samples / 8 cores = 2 samples per
core; weights broadcast. No collectives; gather on host.

Per-core kernel formulation (per sample, C=512 channels, N=1024 spatial):
  h   = groupnorm(x)                (stats via PE indicator matmuls)
  vT  = h^T @ v_w^T                 (spatial on partitions, C free)
  q,k = qkv_w[:1024] @ h            (C on partitions, spatial free)
  s   = k^T q                       (keys j on partitions, queries i free)
  e   = exp(s * C^-0.5)             (logits are O(1); no max-subtraction needed)
  S   = ones^T e                    (softmax denominators via PE reduction)
  o   = vT^T e * (1/S)              (1/S broadcast across partitions via DRAM bounce)
  y   = x + proj_w @ o + proj_b
All matmuls run as float32r (fp32 storage, single-pass PE mode). The two
samples' phases are interleaved in emission order so the second sample's
GroupNorm (DVE) hides under the first sample's attention matmuls (PE).
"""

import numpy as np

import concourse.bass as bass
import concourse.tile as tile
from concourse import bacc, mybir
from concourse.bass_utils import run_bass_kernel_spmd

B, C, H, W = 16, 512, 32, 32
N = H * W              # 1024 spatial positions
G = 32                 # groups
GS = C // G            # 16 channels per group
NCORES = 8
SPC = B // NCORES      # samples per core
EPS = 1e-6
SCALE = float(C) ** -0.5
KT = C // 128          # 4 channel tiles of 128
NT = N // 128          # 8 spatial tiles of 128
NH = N // 512          # 2 free-dim halves of 512

F32 = mybir.dt.float32
F32R = mybir.dt.float32r

_BUILD_CACHE = {}
LAST_RESULT = None  # BassKernelResults of the most recent run (for test harness)


def _build():
    nc = bacc.Bacc("TRN2", target_bir_lowering=False, debug=False)

    x_ext = nc.declare_dram_parameter("x", [SPC, C, N], F32, isOutput=False)
    qkvwT_ext = nc.declare_dram_parameter("qkvwT", [C, 3 * C], F32R, isOutput=False)
    projwT_ext = nc.declare_dram_parameter("projwT", [C, C], F32R, isOutput=False)
    qkvbc_ext = nc.declare_dram_parameter("qkvb_col", [128, 12], F32, isOutput=False)
    cst_ext = nc.declare_dram_parameter("consts12", [128, 12], F32, isOutput=False)
    vbbc_ext = nc.declare_dram_parameter("vb_bc", [128, C], F32, isOutput=False)
    ind_ext = nc.declare_dram_parameter("ind16", [128, 8], F32R, isOutput=False)
    indT_ext = nc.declare_dram_parameter("ind16T", [8, 128], F32R, isOutput=False)
    ones_ext = nc.declare_dram_parameter("ones", [128], F32R, isOutput=False)
    y_ext = nc.declare_dram_parameter("y", [SPC, C, N], F32, isOutput=True)

    sdram = nc.dram_tensor("rs_bounce", [SPC, N], F32)

    Identity = mybir.ActivationFunctionType.Identity
    Exp = mybir.ActivationFunctionType.Exp
    Sqrt = mybir.ActivationFunctionType.Sqrt
    Square = mybir.ActivationFunctionType.Square
    mult = mybir.AluOpType.mult
    add = mybir.AluOpType.add

    with tile.TileContext(nc) as tc:
        with (
            tc.tile_pool(name="wpool", bufs=1) as wpool,
            tc.tile_pool(name="cpool", bufs=1) as cpool,
            tc.tile_pool(name="xpool", bufs=2) as xpool,
            tc.tile_pool(name="hpool", bufs=1) as hpool,
            tc.tile_pool(name="qpool", bufs=1) as qpool,
            tc.tile_pool(name="kpool", bufs=1) as kpool,
            tc.tile_pool(name="vpool", bufs=1) as vpool,
            tc.tile_pool(name="epool", bufs=1) as epool,
            tc.tile_pool(name="opool", bufs=1) as opool,
            tc.tile_pool(name="gnpool", bufs=2) as gnpool,
            tc.tile_pool(name="spool", bufs=1) as spool,
            tc.tile_pool(name="ps", bufs=8, space="PSUM") as ps,
        ):
            # ---- x sample 0 first (256KB chunks across queues) ----
            x_tiles = []
            for s in range(SPC):
                x_tiles.append(
                    xpool.tile([128, KT, N], F32, tag="x", name=f"x_sb{s}")
                )
            for kt in range(KT):
                for sg in range(2):
                    nc.sync.dma_start(
                        out=x_tiles[0][:, kt, sg * 512 : (sg + 1) * 512],
                        in_=x_ext.ap()[
                            0, kt * 128 : (kt + 1) * 128, sg * 512 : (sg + 1) * 512
                        ],
                    )

            # ---- small constants (host-pre-shaped, contiguous loads) ----
            qkvb_col = cpool.tile([128, 12], F32)
            nc.sync.dma_start(out=qkvb_col, in_=qkvbc_ext.ap())
            cst_sb = cpool.tile([128, 12], F32)
            nc.sync.dma_start(out=cst_sb, in_=cst_ext.ap())
            nw_sb = cst_sb[:, 0:4]
            nb_sb = cst_sb[:, 4:8]
            pb_col = cst_sb[:, 8:12]
            vb_bc = cpool.tile([128, C], F32)
            nc.sync.dma_start(out=vb_bc, in_=vbbc_ext.ap())
            ind_sb = cpool.tile([128, 8], F32R)
            nc.sync.dma_start(out=ind_sb, in_=ind_ext.ap())
            indT_sb = cpool.tile([8, 128], F32R)
            nc.sync.dma_start(out=indT_sb, in_=indT_ext.ap())
            ones_col = cpool.tile([128, 1], F32R)
            nc.sync.dma_start(out=ones_col, in_=ones_ext.ap().unsqueeze(1))
            eps_sb = cpool.tile([8, 1], F32)
            nc.vector.memset(eps_sb, EPS)
            warm_sb = cpool.tile([8, 1], F32)
            nc.scalar.activation(out=warm_sb, in_=eps_sb, func=Identity)
            nc.scalar.activation(out=warm_sb, in_=eps_sb, func=Sqrt)
            nc.scalar.activation(out=warm_sb, in_=eps_sb, func=Exp)

            # ---- weights (v columns first per kt), then x1, then proj ----
            qkvw_sb = wpool.tile([128, KT, 3 * C], F32R)
            projw_sb = wpool.tile([128, KT, C], F32R)
            for kt in range(KT):
                for chunk in (2, 0, 1):  # v, q, k column blocks
                    nc.sync.dma_start(
                        out=qkvw_sb[:, kt, chunk * C : (chunk + 1) * C],
                        in_=qkvwT_ext.ap()[
                            kt * 128 : (kt + 1) * 128, chunk * C : (chunk + 1) * C
                        ],
                    )
            for kt in range(KT):
                for sg in range(2):
                    nc.sync.dma_start(
                        out=x_tiles[1][:, kt, sg * 512 : (sg + 1) * 512],
                        in_=x_ext.ap()[
                            1, kt * 128 : (kt + 1) * 128, sg * 512 : (sg + 1) * 512
                        ],
                    )
            for kt in range(KT):
                nc.sync.dma_start(
                    out=projw_sb[:, kt, :],
                    in_=projwT_ext.ap()[kt * 128 : (kt + 1) * 128, :],
                )

            def gn_stats(s, act_split):
                """GroupNorm stats for sample s -> mr (8,KT,2) F32R [mean,rstd].

                act_split: compute kt2/kt3 moments on ScalarE (Identity/Square
                with accum_out) so the DVE and ACT halves run concurrently —
                used for sample 0 where head latency is PE-gating.
                """
                x_sb = x_tiles[s]
                ps_gs = ps.tile([8, KT, 2], F32, tag="mm", name=f"ps_gs{s}")
                dve_kts = (0, 1) if act_split else (0, 1, 2, 3)
                for kt in dve_kts:
                    stats = gnpool.tile(
                        [128, 2, 6], F32, tag=f"stats{kt}", name=f"stats{s}_{kt}"
                    )
                    for sg in range(2):
                        nc.vector.bn_stats(
                            out=stats[:, sg, :],
                            in_=x_sb[:, kt, sg * 512 : (sg + 1) * 512],
                        )
                    mv = gnpool.tile([128, 2], F32, tag=f"mv{kt}", name=f"mv{s}_{kt}")
                    nc.vector.bn_aggr(out=mv, in_=stats)
                    s2f = gnpool.tile(
                        [128, 2], F32, tag=f"s2f{kt}", name=f"s2f{s}_{kt}"
                    )
                    # [E[x], E[x^2]] = [mean, var + mean^2]
                    nc.vector.tensor_mul(s2f[:, 1:2], mv[:, 0:1], mv[:, 0:1])
                    nc.vector.tensor_add(s2f[:, 1:2], s2f[:, 1:2], mv[:, 1:2])
                    nc.vector.tensor_copy(s2f[:, 0:1], mv[:, 0:1])
                    s2 = gnpool.tile(
                        [128, 2], F32R, tag=f"s2_{kt}", name=f"s2_{s}_{kt}"
                    )
                    nc.vector.tensor_copy(s2, s2f)
                    nc.tensor.matmul(
                        ps_gs[:, kt, :], ind_sb, s2, start=True, stop=True
                    )
                if act_split:
                    for kt in (2, 3):
                        s2f = gnpool.tile(
                            [128, 2], F32, tag=f"s2f{kt}", name=f"s2f{s}_{kt}"
                        )
                        scr = gnpool.tile(
                            [128, N], F32, tag="gnscr", name=f"scr{s}_{kt}"
                        )
                        # E[x] and E[x^2] via ScalarE accumulators (exact pwp)
                        nc.scalar.activation(
                            out=scr, in_=x_sb[:, kt, :], func=Identity,
                            scale=1.0 / N, accum_out=s2f[:, 0:1],
                        )
                        scr2 = gnpool.tile(
                            [128, N], F32, tag="gnscr", name=f"scr2{s}_{kt}"
                        )
                        nc.scalar.activation(
                            out=scr2, in_=x_sb[:, kt, :], func=Square,
                            scale=N ** -0.5, accum_out=s2f[:, 1:2],
                        )
                        s2 = gnpool.tile(
                            [128, 2], F32R, tag=f"s2_{kt}", name=f"s2_{s}_{kt}"
                        )
                        nc.vector.tensor_copy(s2, s2f)
                        nc.tensor.matmul(
                            ps_gs[:, kt, :], ind_sb, s2, start=True, stop=True
                        )
                # group mean / E[x^2] (divide by 16 partitions per group)
                gs = gnpool.tile([8, KT, 2], F32, tag="gs", name=f"gs{s}")
                nc.vector.tensor_scalar_mul(gs, ps_gs, 1.0 / GS)
                msq = gnpool.tile([8, KT], F32, tag="msq", name=f"msq{s}")
                nc.vector.tensor_mul(msq, gs[:, :, 0], gs[:, :, 0])
                nc.vector.tensor_sub(gs[:, :, 1], gs[:, :, 1], msq)
                nc.scalar.activation(
                    out=gs[:, :, 1], in_=gs[:, :, 1], func=Sqrt, bias=eps_sb
                )
                nc.vector.reciprocal(gs[:, :, 1], gs[:, :, 1])
                # rounded copy [mean, rstd] feeding the broadcast matmul
                mr = gnpool.tile([8, KT, 2], F32R, tag="mr", name=f"mr{s}")
                nc.vector.tensor_copy(mr, gs)
                return mr

            def gn_apply(s, mr, engines):
                """Broadcast stats to channels and apply x*scale+bias -> h."""
                x_sb = x_tiles[s]
                h_sb = hpool.tile([128, KT, N], F32R, tag="h", name=f"h{s}")
                for kt in range(KT):
                    ps_bc = ps.tile([128, 2], F32, tag="mm", name=f"ps_bc{s}_{kt}")
                    nc.tensor.matmul(
                        ps_bc, indT_sb, mr[:, kt, :], start=True, stop=True
                    )
                    scb = gnpool.tile(
                        [128, 2], F32, tag=f"scb{kt}", name=f"scb{s}_{kt}"
                    )
                    nc.vector.tensor_mul(
                        scb[:, 0:1], ps_bc[:, 1:2], nw_sb[:, kt : kt + 1]
                    )
                    nc.vector.tensor_mul(scb[:, 1:2], ps_bc[:, 0:1], scb[:, 0:1])
                    nc.vector.tensor_sub(
                        scb[:, 1:2], nb_sb[:, kt : kt + 1], scb[:, 1:2]
                    )
                    if engines[kt] == "v":
                        nc.vector.tensor_scalar(
                            out=h_sb[:, kt, :],
                            in0=x_sb[:, kt, :],
                            scalar1=scb[:, 0:1],
                            scalar2=scb[:, 1:2],
                            op0=mult,
                            op1=add,
                        )
                    else:
                        nc.scalar.activation(
                            out=h_sb[:, kt, :], in_=x_sb[:, kt, :],
                            func=Identity, scale=scb[:, 0:1], bias=scb[:, 1:2],
                        )
                return h_sb

            def v_transposed(h_sb):
                """vT = h^T @ v_w^T (+ v bias broadcast along free dim)."""
                vT_sb = vpool.tile([128, NT, C], F32R, tag="vT")
                for nt in range(NT):
                    pm = ps.tile([128, 512], F32, tag="mm")
                    for kt in range(KT):
                        nc.tensor.matmul(
                            pm,
                            h_sb[:, kt, nt * 128 : (nt + 1) * 128],
                            qkvw_sb[:, kt, 2 * C : 3 * C],
                            start=(kt == 0),
                            stop=(kt == KT - 1),
                        )
                    nc.vector.tensor_add(vT_sb[:, nt, :], pm, vb_bc)
                return vT_sb

            def qk(h_sb):
                q_sb = qpool.tile([128, KT, N], F32R, tag="q")
                k_sb = kpool.tile([128, KT, N], F32R, tag="k")
                for ih in range(NH):
                    for ot in range(8):
                        dest = q_sb if ot < 4 else k_sb
                        oc = ot % 4
                        pm = ps.tile([128, 512], F32, tag="mm")
                        for kt in range(KT):
                            nc.tensor.matmul(
                                pm,
                                qkvw_sb[:, kt, ot * 128 : (ot + 1) * 128],
                                h_sb[:, kt, ih * 512 : (ih + 1) * 512],
                                start=(kt == 0),
                                stop=(kt == KT - 1),
                            )
                        nc.scalar.activation(
                            out=dest[:, oc, ih * 512 : (ih + 1) * 512],
                            in_=pm,
                            func=Identity,
                            bias=qkvb_col[:, ot : ot + 1],
                        )
                return q_sb, k_sb

            def attn_scores(s, q_sb, k_sb):
                # s = k^T q (keys on partitions); e = exp(s * scale)
                e_sb = epool.tile([128, NT, N], F32R, tag="e", name=f"e{s}")
                for jt in range(NT):
                    for ih in range(NH):
                        pm = ps.tile([128, 512], F32, tag="mm")
                        for ot in range(KT):
                            nc.tensor.matmul(
                                pm,
                                k_sb[:, ot, jt * 128 : (jt + 1) * 128],
                                q_sb[:, ot, ih * 512 : (ih + 1) * 512],
                                start=(ot == 0),
                                stop=(ot == KT - 1),
                            )
                        nc.scalar.activation(
                            out=e_sb[:, jt, ih * 512 : (ih + 1) * 512],
                            in_=pm,
                            func=Exp,
                            scale=SCALE,
                        )
                return e_sb

            def attn_out(s, e_sb, vT_sb):
                # softmax denominators S = sum_j e; 1/S broadcast via DRAM
                recipS = spool.tile([1, N], F32, tag="recipS", name=f"recipS{s}")
                for ih in range(NH):
                    pS = ps.tile([1, 512], F32, tag="mm")
                    for jt in range(NT):
                        nc.tensor.matmul(
                            pS,
                            ones_col,
                            e_sb[:, jt, ih * 512 : (ih + 1) * 512],
                            start=(jt == 0),
                            stop=(jt == NT - 1),
                        )
                    nc.vector.reciprocal_approx_fast(
                        out=recipS[:, ih * 512 : (ih + 1) * 512], in_=pS
                    )
                nc.sync.dma_start(out=sdram.ap()[s].unsqueeze(0), in_=recipS)
                rSbc = spool.tile([128, N], F32, tag="rSbc", name=f"rSbc{s}")
                nc.sync.dma_start(
                    out=rSbc, in_=sdram.ap()[s].partition_broadcast(128)
                )
                # o = vT^T @ e, normalized by 1/S
                o_sb = opool.tile([128, KT, N], F32R, tag="o", name=f"o{s}")
                for ct in range(KT):
                    for ih in range(NH):
                        pm = ps.tile([128, 512], F32, tag="mm")
                        for jt in range(NT):
                            nc.tensor.matmul(
                                pm,
                                vT_sb[:, jt, ct * 128 : (ct + 1) * 128],
                                e_sb[:, jt, ih * 512 : (ih + 1) * 512],
                                start=(jt == 0),
                                stop=(jt == NT - 1),
                            )
                        nc.vector.tensor_mul(
                            o_sb[:, ct, ih * 512 : (ih + 1) * 512],
                            pm,
                            rSbc[:, ih * 512 : (ih + 1) * 512],
                        )
                return o_sb

            def proj_resid(s, o_sb, x_sb):
                # residual accumulates in place into the (now dead) x tile
                for ct2 in range(KT):
                    for ih in range(NH):
                        pm = ps.tile([128, 512], F32, tag="mm")
                        for ckt in range(KT):
                            nc.tensor.matmul(
                                pm,
                                projw_sb[:, ckt, ct2 * 128 : (ct2 + 1) * 128],
                                o_sb[:, ckt, ih * 512 : (ih + 1) * 512],
                                start=(ckt == 0),
                                stop=(ckt == KT - 1),
                            )
                        # + proj bias, in place on PSUM (ScalarE)
                        nc.scalar.activation(
                            out=pm, in_=pm, func=Identity,
                            bias=pb_col[:, ct2 : ct2 + 1],
                        )
                        # + residual, in place into x
                        nc.vector.tensor_add(
                            x_sb[:, ct2, ih * 512 : (ih + 1) * 512],
                            pm,
                            x_sb[:, ct2, ih * 512 : (ih + 1) * 512],
                        )
                        nc.gpsimd.dma_start(
                            out=y_ext.ap()[
                                s,
                                ct2 * 128 : (ct2 + 1) * 128,
                                ih * 512 : (ih + 1) * 512,
                            ],
                            in_=x_sb[:, ct2, ih * 512 : (ih + 1) * 512],
                        )

            # ---- interleaved two-sample schedule ----
            mr0 = gn_stats(0, act_split=False)
            h0 = gn_apply(0, mr0, engines="avav")
            vT0 = v_transposed(h0)
            q0, k0 = qk(h0)
            mr1 = gn_stats(1, act_split=False)  # DVE-only, hides under s0 attn
            e0 = attn_scores(0, q0, k0)
            h1 = gn_apply(1, mr1, engines="aaaa")  # ACT applies, DVE stays free
            o0 = attn_out(0, e0, vT0)
            proj_resid(0, o0, x_tiles[0])
            vT1 = v_transposed(h1)
            q1, k1 = qk(h1)
            e1 = attn_scores(1, q1, k1)
            o1 = attn_out(1, e1, vT1)
            proj_resid(1, o1, x_tiles[1])

    nc.compile()
    return nc


def _get_nc():
    if "nc" not in _BUILD_CACHE:
        _BUILD_CACHE["nc"] = _build()
    return _BUILD_CACHE["nc"]


def kernel(x, norm_w, norm_b, qkv_w, qkv_b, proj_w, proj_b, _trace=False):
    global LAST_RESULT
    nc = _get_nc()

    x = np.asarray(x, dtype=np.float32).reshape(B, C, N)
    qkvwT = np.ascontiguousarray(np.asarray(qkv_w, dtype=np.float32).T)
    projwT = np.ascontiguousarray(np.asarray(proj_w, dtype=np.float32).T)
    ind16 = np.zeros((128, 8), dtype=np.float32)
    for p in range(128):
        ind16[p, p // GS] = 1.0
    ind16T = np.ascontiguousarray(ind16.T)

    norm_w = np.asarray(norm_w, dtype=np.float32)
    norm_b = np.asarray(norm_b, dtype=np.float32)
    qkv_b = np.asarray(qkv_b, dtype=np.float32)
    proj_b = np.asarray(proj_b, dtype=np.float32)
    # per-o-tile bias columns: col t holds bias[t*128 : (t+1)*128]
    qkvb_col = np.ascontiguousarray(qkv_b.reshape(12, 128).T)
    consts12 = np.ascontiguousarray(
        np.concatenate(
            [
                norm_w.reshape(KT, 128).T,
                norm_b.reshape(KT, 128).T,
                proj_b.reshape(KT, 128).T,
            ],
            axis=1,
        )
    )
    vb_bc = np.ascontiguousarray(
        np.broadcast_to(qkv_b[2 * C : 3 * C], (128, C))
    )
    shared = {
        "qkvwT": qkvwT,
        "projwT": projwT,
        "qkvb_col": qkvb_col,
        "consts12": consts12,
        "vb_bc": vb_bc,
        "ind16": ind16,
        "ind16T": ind16T,
        "ones": np.ones(128, dtype=np.float32),
    }
    in_maps = [
        {"x": np.ascontiguousarray(x[c * SPC : (c + 1) * SPC]), **shared}
        for c in range(NCORES)
    ]
    res = run_bass_kernel_spmd(nc, in_maps, list(range(NCORES)), trace=_trace)
    LAST_RESULT = res
    out = np.concatenate([res.results[i]["y"] for i in range(NCORES)], axis=0)
    return out.reshape(B, C, H, W)


# revision 38
# speedup vs baseline: 1.0015x; 1.0015x over previous
"""AttnBlock (GroupNorm + 1x1-conv QKV + single-head spatial attention + proj
+ residual) on 8 Trainium2 NeuronCores.

Sharding: pure data-parallel over batch — 16 samples / 8 cores = 2 samples per
core; weights broadcast. No collectives; gather on host.

Per-core kernel formulation (per sample, C=512 channels, N=1024 spatial):
  h   = groupnorm(x)                (stats via PE indicator matmuls)
  vT  = h^T @ v_w^T                 (spatial on partitions, C free)
  q,k = qkv_w[:1024] @ h            (C on partitions, spatial free)
  s   = k^T q                       (keys j on partitions, queries i free)
  e   = exp(s * C^-0.5)             (logits are O(1); no max-subtraction needed)
  S   = ones^T e                    (softmax denominators via PE reduction)
  o   = vT^T e * (1/S)              (1/S broadcast across partitions via DRAM bounce)
  y   = x + proj_w @ o + proj_b
All matmuls run as float32r (fp32 storage, single-pass PE mode). The two
samples' phases are interleaved in emission order so the second sample's
GroupNorm (DVE) hides under the first sample's attention matmuls (PE).
"""

import numpy as np

import concourse.bass as bass
import concourse.tile as tile
from concourse import bacc, mybir
from concourse.bass_utils import run_bass_kernel_spmd

B, C, H, W = 16, 512, 32, 32
N = H * W              # 1024 spatial positions
G = 32                 # groups
GS = C // G            # 16 channels per group
NCORES = 8
SPC = B // NCORES      # samples per core
EPS = 1e-6
SCALE = float(C) ** -0.5
KT = C // 128          # 4 channel tiles of 128
NT = N // 128          # 8 spatial tiles of 128
NH = N // 512          # 2 free-dim halves of 512

F32 = mybir.dt.float32
F32R = mybir.dt.float32r

_BUILD_CACHE = {}
LAST_RESULT = None  # BassKernelResults of the most recent run (for test harness)


def _build():
    nc = bacc.Bacc("TRN2", target_bir_lowering=False, debug=False, num_swdge_queues=4)

    x_ext = nc.declare_dram_parameter("x", [SPC, C, N], F32, isOutput=False)
    qkvwT_ext = nc.declare_dram_parameter("qkvwT", [C, 3 * C], F32R, isOutput=False)
    projwT_ext = nc.declare_dram_parameter("projwT", [C, C], F32R, isOutput=False)
    qkvbc_ext = nc.declare_dram_parameter("qkvb_col", [128, 12], F32, isOutput=False)
    cst_ext = nc.declare_dram_parameter("consts12", [128, 12], F32, isOutput=False)
    vbbc_ext = nc.declare_dram_parameter("vb_bc", [128, C], F32, isOutput=False)
    ind_ext = nc.declare_dram_parameter("ind16", [128, 8], F32R, isOutput=False)
    indT_ext = nc.declare_dram_parameter("ind16T", [8, 128], F32R, isOutput=False)
    ones_ext = nc.declare_dram_parameter("ones", [128], F32R, isOutput=False)
    y_ext = nc.declare_dram_parameter("y", [SPC, C, N], F32, isOutput=True)

    sdram = nc.dram_tensor("rs_bounce", [SPC, N], F32)

    Identity = mybir.ActivationFunctionType.Identity
    Exp = mybir.ActivationFunctionType.Exp
    Sqrt = mybir.ActivationFunctionType.Sqrt
    Square = mybir.ActivationFunctionType.Square
    mult = mybir.AluOpType.mult
    add = mybir.AluOpType.add

    with tile.TileContext(nc) as tc:
        with (
            tc.tile_pool(name="wpool", bufs=1) as wpool,
            tc.tile_pool(name="cpool", bufs=1) as cpool,
            tc.tile_pool(name="xpool", bufs=2) as xpool,
            tc.tile_pool(name="hpool", bufs=1) as hpool,
            tc.tile_pool(name="qpool", bufs=1) as qpool,
            tc.tile_pool(name="kpool", bufs=1) as kpool,
            tc.tile_pool(name="vpool", bufs=1) as vpool,
            tc.tile_pool(name="epool", bufs=1) as epool,
            tc.tile_pool(name="opool", bufs=1) as opool,
            tc.tile_pool(name="gnpool", bufs=2) as gnpool,
            tc.tile_pool(name="spool", bufs=1) as spool,
            tc.tile_pool(name="ps", bufs=8, space="PSUM") as ps,
        ):
            # ---- x sample 0 first (256KB chunks across queues) ----
            x_tiles = []
            for s in range(SPC):
                x_tiles.append(
                    xpool.tile([128, KT, N], F32, tag="x", name=f"x_sb{s}")
                )
            for kt in range(KT):
                for sg in range(2):
                    nc.sync.dma_start(
                        out=x_tiles[0][:, kt, sg * 512 : (sg + 1) * 512],
                        in_=x_ext.ap()[
                            0, kt * 128 : (kt + 1) * 128, sg * 512 : (sg + 1) * 512
                        ],
                    )

            # ---- small constants (host-pre-shaped, contiguous loads) ----
            qkvb_col = cpool.tile([128, 12], F32)
            nc.sync.dma_start(out=qkvb_col, in_=qkvbc_ext.ap())
            cst_sb = cpool.tile([128, 12], F32)
            nc.sync.dma_start(out=cst_sb, in_=cst_ext.ap())
            nw_sb = cst_sb[:, 0:4]
            nb_sb = cst_sb[:, 4:8]
            pb_col = cst_sb[:, 8:12]
            vb_bc = cpool.tile([128, C], F32)
            nc.sync.dma_start(out=vb_bc, in_=vbbc_ext.ap())
            ind_sb = cpool.tile([128, 8], F32R)
            nc.sync.dma_start(out=ind_sb, in_=ind_ext.ap())
            indT_sb = cpool.tile([8, 128], F32R)
            nc.sync.dma_start(out=indT_sb, in_=indT_ext.ap())
            ones_col = cpool.tile([128, 1], F32R)
            nc.sync.dma_start(out=ones_col, in_=ones_ext.ap().unsqueeze(1))
            eps_sb = cpool.tile([8, 1], F32)
            nc.vector.memset(eps_sb, EPS)
            warm_sb = cpool.tile([8, 1], F32)
            nc.scalar.activation(out=warm_sb, in_=eps_sb, func=Identity)
            nc.scalar.activation(out=warm_sb, in_=eps_sb, func=Sqrt)
            nc.scalar.activation(out=warm_sb, in_=eps_sb, func=Exp)

            # ---- weights (v columns first per kt), then x1, then proj ----
            qkvw_sb = wpool.tile([128, KT, 3 * C], F32R)
            projw_sb = wpool.tile([128, KT, C], F32R)
            for kt in range(KT):
                for chunk in (2, 0, 1):  # v, q, k column blocks
                    nc.sync.dma_start(
                        out=qkvw_sb[:, kt, chunk * C : (chunk + 1) * C],
                        in_=qkvwT_ext.ap()[
                            kt * 128 : (kt + 1) * 128, chunk * C : (chunk + 1) * C
                        ],
                    )
            for kt in range(KT):
                for sg in range(2):
                    nc.sync.dma_start(
                        out=x_tiles[1][:, kt, sg * 512 : (sg + 1) * 512],
                        in_=x_ext.ap()[
                            1, kt * 128 : (kt + 1) * 128, sg * 512 : (sg + 1) * 512
                        ],
                    )
            for kt in range(KT):
                nc.sync.dma_start(
                    out=projw_sb[:, kt, :],
                    in_=projwT_ext.ap()[kt * 128 : (kt + 1) * 128, :],
                )

            def gn_stats(s, act_split):
                """GroupNorm stats for sample s -> mr (8,KT,2) F32R [mean,rstd].

                act_split: compute kt2/kt3 moments on ScalarE (Identity/Square
                with accum_out) so the DVE and ACT halves run concurrently —
                used for sample 0 where head latency is PE-gating.
                """
                x_sb = x_tiles[s]
                ps_gs = ps.tile([8, KT, 2], F32, tag="mm", name=f"ps_gs{s}")
                dve_kts = (0, 1) if act_split else (0, 1, 2, 3)
                for kt in dve_kts:
                    stats = gnpool.tile(
                        [128, 2, 6], F32, tag=f"stats{kt}", name=f"stats{s}_{kt}"
                    )
                    for sg in range(2):
                        nc.vector.bn_stats(
                            out=stats[:, sg, :],
                            in_=x_sb[:, kt, sg * 512 : (sg + 1) * 512],
                        )
                    mv = gnpool.tile([128, 2], F32, tag=f"mv{kt}", name=f"mv{s}_{kt}")
                    nc.vector.bn_aggr(out=mv, in_=stats)
                    s2f = gnpool.tile(
                        [128, 2], F32, tag=f"s2f{kt}", name=f"s2f{s}_{kt}"
                    )
                    # [E[x], E[x^2]] = [mean, var + mean^2]
                    nc.vector.tensor_mul(s2f[:, 1:2], mv[:, 0:1], mv[:, 0:1])
                    nc.vector.tensor_add(s2f[:, 1:2], s2f[:, 1:2], mv[:, 1:2])
                    nc.vector.tensor_copy(s2f[:, 0:1], mv[:, 0:1])
                    s2 = gnpool.tile(
                        [128, 2], F32R, tag=f"s2_{kt}", name=f"s2_{s}_{kt}"
                    )
                    nc.vector.tensor_copy(s2, s2f)
                    nc.tensor.matmul(
                        ps_gs[:, kt, :], ind_sb, s2, start=True, stop=True
                    )
                if act_split:
                    for kt in (2, 3):
                        s2f = gnpool.tile(
                            [128, 2], F32, tag=f"s2f{kt}", name=f"s2f{s}_{kt}"
                        )
                        scr = gnpool.tile(
                            [128, N], F32, tag="gnscr", name=f"scr{s}_{kt}"
                        )
                        # E[x] and E[x^2] via ScalarE accumulators (exact pwp)
                        nc.scalar.activation(
                            out=scr, in_=x_sb[:, kt, :], func=Identity,
                            scale=1.0 / N, accum_out=s2f[:, 0:1],
                        )
                        scr2 = gnpool.tile(
                            [128, N], F32, tag="gnscr", name=f"scr2{s}_{kt}"
                        )
                        nc.scalar.activation(
                            out=scr2, in_=x_sb[:, kt, :], func=Square,
                            scale=N ** -0.5, accum_out=s2f[:, 1:2],
                        )
                        s2 = gnpool.tile(
                            [128, 2], F32R, tag=f"s2_{kt}", name=f"s2_{s}_{kt}"
                        )
                        nc.vector.tensor_copy(s2, s2f)
                        nc.tensor.matmul(
                            ps_gs[:, kt, :], ind_sb, s2, start=True, stop=True
                        )
                # group mean / E[x^2] (divide by 16 partitions per group)
                gs = gnpool.tile([8, KT, 2], F32, tag="gs", name=f"gs{s}")
                nc.vector.tensor_scalar_mul(gs, ps_gs, 1.0 / GS)
                msq = gnpool.tile([8, KT], F32, tag="msq", name=f"msq{s}")
                nc.vector.tensor_mul(msq, gs[:, :, 0], gs[:, :, 0])
                nc.vector.tensor_sub(gs[:, :, 1], gs[:, :, 1], msq)
                nc.scalar.activation(
                    out=gs[:, :, 1], in_=gs[:, :, 1], func=Sqrt, bias=eps_sb
                )
                nc.vector.reciprocal(gs[:, :, 1], gs[:, :, 1])
                # rounded copy [mean, rstd] feeding the broadcast matmul
                mr = gnpool.tile([8, KT, 2], F32R, tag="mr", name=f"mr{s}")
                nc.vector.tensor_copy(mr, gs)
                return mr

            def gn_apply(s, mr, engines):
                """Broadcast stats to channels and apply x*scale+bias -> h."""
                x_sb = x_tiles[s]
                h_sb = hpool.tile([128, KT, N], F32R, tag="h", name=f"h{s}")
                for kt in range(KT):
                    ps_bc = ps.tile([128, 2], F32, tag="mm", name=f"ps_bc{s}_{kt}")
                    nc.tensor.matmul(
                        ps_bc, indT_sb, mr[:, kt, :], start=True, stop=True
                    )
                    scb = gnpool.tile(
                        [128, 2], F32, tag=f"scb{kt}", name=f"scb{s}_{kt}"
                    )
                    nc.vector.tensor_mul(
                        scb[:, 0:1], ps_bc[:, 1:2], nw_sb[:, kt : kt + 1]
                    )
                    nc.vector.tensor_mul(scb[:, 1:2], ps_bc[:, 0:1], scb[:, 0:1])
                    nc.vector.tensor_sub(
                        scb[:, 1:2], nb_sb[:, kt : kt + 1], scb[:, 1:2]
                    )
                    if engines[kt] == "v":
                        nc.vector.tensor_scalar(
                            out=h_sb[:, kt, :],
                            in0=x_sb[:, kt, :],
                            scalar1=scb[:, 0:1],
                            scalar2=scb[:, 1:2],
                            op0=mult,
                            op1=add,
                        )
                    else:
                        nc.scalar.activation(
                            out=h_sb[:, kt, :], in_=x_sb[:, kt, :],
                            func=Identity, scale=scb[:, 0:1], bias=scb[:, 1:2],
                        )
                return h_sb

            def v_transposed(h_sb):
                """vT = h^T @ v_w^T (+ v bias broadcast along free dim)."""
                vT_sb = vpool.tile([128, NT, C], F32R, tag="vT")
                for nt in range(NT):
                    pm = ps.tile([128, 512], F32, tag="mm")
                    for kt in range(KT):
                        nc.tensor.matmul(
                            pm,
                            h_sb[:, kt, nt * 128 : (nt + 1) * 128],
                            qkvw_sb[:, kt, 2 * C : 3 * C],
                            start=(kt == 0),
                            stop=(kt == KT - 1),
                        )
                    nc.vector.tensor_add(vT_sb[:, nt, :], pm, vb_bc)
                return vT_sb

            def qk(h_sb):
                q_sb = qpool.tile([128, KT, N], F32R, tag="q")
                k_sb = kpool.tile([128, KT, N], F32R, tag="k")
                for ih in range(NH):
                    for ot in range(8):
                        dest = q_sb if ot < 4 else k_sb
                        oc = ot % 4
                        pm = ps.tile([128, 512], F32, tag="mm")
                        for kt in range(KT):
                            nc.tensor.matmul(
                                pm,
                                qkvw_sb[:, kt, ot * 128 : (ot + 1) * 128],
                                h_sb[:, kt, ih * 512 : (ih + 1) * 512],
                                start=(kt == 0),
                                stop=(kt == KT - 1),
                            )
                        nc.scalar.activation(
                            out=dest[:, oc, ih * 512 : (ih + 1) * 512],
                            in_=pm,
                            func=Identity,
                            bias=qkvb_col[:, ot : ot + 1],
                        )
                return q_sb, k_sb

            def attn_scores(s, q_sb, k_sb):
                # s = k^T q (keys on partitions); e = exp(s * scale)
                e_sb = epool.tile([128, NT, N], F32R, tag="e", name=f"e{s}")
                for jt in range(NT):
                    for ih in range(NH):
                        pm = ps.tile([128, 512], F32, tag="mm")
                        for ot in range(KT):
                            nc.tensor.matmul(
                                pm,
                                k_sb[:, ot, jt * 128 : (jt + 1) * 128],
                                q_sb[:, ot, ih * 512 : (ih + 1) * 512],
                                start=(ot == 0),
                                stop=(ot == KT - 1),
                            )
                        nc.scalar.activation(
                            out=e_sb[:, jt, ih * 512 : (ih + 1) * 512],
                            in_=pm,
                            func=Exp,
                            scale=SCALE,
                        )
                return e_sb

            def attn_out(s, e_sb, vT_sb):
                # softmax denominators S = sum_j e; 1/S broadcast via DRAM
                recipS = spool.tile([1, N], F32, tag="recipS", name=f"recipS{s}")
                for ih in range(NH):
                    pS = ps.tile([1, 512], F32, tag="mm")
                    for jt in range(NT):
                        nc.tensor.matmul(
                            pS,
                            ones_col,
                            e_sb[:, jt, ih * 512 : (ih + 1) * 512],
                            start=(jt == 0),
                            stop=(jt == NT - 1),
                        )
                    nc.vector.reciprocal_approx_fast(
                        out=recipS[:, ih * 512 : (ih + 1) * 512], in_=pS
                    )
                nc.sync.dma_start(out=sdram.ap()[s].unsqueeze(0), in_=recipS)
                rSbc = spool.tile([128, N], F32, tag="rSbc", name=f"rSbc{s}")
                nc.sync.dma_start(
                    out=rSbc, in_=sdram.ap()[s].partition_broadcast(128)
                )
                # o = vT^T @ e, normalized by 1/S
                o_sb = opool.tile([128, KT, N], F32R, tag="o", name=f"o{s}")
                for ct in range(KT):
                    for ih in range(NH):
                        pm = ps.tile([128, 512], F32, tag="mm")
                        for jt in range(NT):
                            nc.tensor.matmul(
                                pm,
                                vT_sb[:, jt, ct * 128 : (ct + 1) * 128],
                                e_sb[:, jt, ih * 512 : (ih + 1) * 512],
                                start=(jt == 0),
                                stop=(jt == NT - 1),
                            )
                        nc.vector.tensor_mul(
                            o_sb[:, ct, ih * 512 : (ih + 1) * 512],
                            pm,
                            rSbc[:, ih * 512 : (ih + 1) * 512],
                        )
                return o_sb

            def proj_resid(s, o_sb, x_sb):
                # residual accumulates in place into the (now dead) x tile
                for ct2 in range(KT):
                    for ih in range(NH):
                        pm = ps.tile([128, 512], F32, tag="mm")
                        for ckt in range(KT):
                            nc.tensor.matmul(
                                pm,
                                projw_sb[:, ckt, ct2 * 128 : (ct2 + 1) * 128],
                                o_sb[:, ckt, ih * 512 : (ih + 1) * 512],
                                start=(ckt == 0),
                                stop=(ckt == KT - 1),
                            )
                        # + proj bias, in place on PSUM (ScalarE)
                        nc.scalar.activation(
                            out=pm, in_=pm, func=Identity,
                            bias=pb_col[:, ct2 : ct2 + 1],
                        )
                        # + residual, in place into x
                        nc.vector.tensor_add(
                            x_sb[:, ct2, ih * 512 : (ih + 1) * 512],
                            pm,
                            x_sb[:, ct2, ih * 512 : (ih + 1) * 512],
                        )
                        nc.gpsimd.dma_start(
                            out=y_ext.ap()[
                                s,
                                ct2 * 128 : (ct2 + 1) * 128,
                                ih * 512 : (ih + 1) * 512,
                            ],
                            in_=x_sb[:, ct2, ih * 512 : (ih + 1) * 512],
                        )

            # ---- interleaved two-sample schedule ----
            mr0 = gn_stats(0, act_split=False)
            h0 = gn_apply(0, mr0, engines="avav")
            vT0 = v_transposed(h0)
            q0, k0 = qk(h0)
            mr1 = gn_stats(1, act_split=False)  # DVE-only, hides under s0 attn
            e0 = attn_scores(0, q0, k0)
            h1 = gn_apply(1, mr1, engines="aaaa")  # ACT applies, DVE stays free
            o0 = attn_out(0, e0, vT0)
            proj_resid(0, o0, x_tiles[0])
            vT1 = v_transposed(h1)
            q1, k1 = qk(h1)
            e1 = attn_scores(1, q1, k1)
            o1 = attn_out(1, e1, vT1)
            proj_resid(1, o1, x_tiles[1])

    nc.compile()
    return nc


def _get_nc():
    if "nc" not in _BUILD_CACHE:
        _BUILD_CACHE["nc"] = _build()
    return _BUILD_CACHE["nc"]


def kernel(x, norm_w, norm_b, qkv_w, qkv_b, proj_w, proj_b, _trace=False):
    global LAST_RESULT
    nc = _get_nc()

    x = np.asarray(x, dtype=np.float32).reshape(B, C, N)
    qkvwT = np.ascontiguousarray(np.asarray(qkv_w, dtype=np.float32).T)
    projwT = np.ascontiguousarray(np.asarray(proj_w, dtype=np.float32).T)
    ind16 = np.zeros((128, 8), dtype=np.float32)
    for p in range(128):
        ind16[p, p // GS] = 1.0
    ind16T = np.ascontiguousarray(ind16.T)

    norm_w = np.asarray(norm_w, dtype=np.float32)
    norm_b = np.asarray(norm_b, dtype=np.float32)
    qkv_b = np.asarray(qkv_b, dtype=np.float32)
    proj_b = np.asarray(proj_b, dtype=np.float32)
    # per-o-tile bias columns: col t holds bias[t*128 : (t+1)*128]
    qkvb_col = np.ascontiguousarray(qkv_b.reshape(12, 128).T)
    consts12 = np.ascontiguousarray(
        np.concatenate(
            [
                norm_w.reshape(KT, 128).T,
                norm_b.reshape(KT, 128).T,
                proj_b.reshape(KT, 128).T,
            ],
            axis=1,
        )
    )
    vb_bc = np.ascontiguousarray(
        np.broadcast_to(qkv_b[2 * C : 3 * C], (128, C))
    )
    shared = {
        "qkvwT": qkvwT,
        "projwT": projwT,
        "qkvb_col": qkvb_col,
        "consts12": consts12,
        "vb_bc": vb_bc,
        "ind16": ind16,
        "ind16T": ind16T,
        "ones": np.ones(128, dtype=np.float32),
    }
    in_maps = [
        {"x": np.ascontiguousarray(x[c * SPC : (c + 1) * SPC]), **shared}
        for c in range(NCORES)
    ]
    res = run_bass_kernel_spmd(nc, in_maps, list(range(NCORES)), trace=_trace)
    LAST_RESULT = res
    out = np.concatenate([res.results[i]["y"] for i in range(NCORES)], axis=0)
    return out.reshape(B, C, H, W)


# revision 39
# speedup vs baseline: 1.0139x; 1.0124x over previous
"""AttnBlock (GroupNorm + 1x1-conv QKV + single-head spatial attention + proj
+ residual) on 8 Trainium2 NeuronCores.

Sharding: pure data-parallel over batch — 16 samples / 8 cores = 2 samples per
core; weights broadcast. No collectives; gather on host.

Per-core kernel formulation (per sample, C=512 channels, N=1024 spatial):
  h   = groupnorm(x)                (stats via PE indicator matmuls)
  vT  = h^T @ v_w^T                 (spatial on partitions, C free)
  q,k = qkv_w[:1024] @ h            (C on partitions, spatial free)
  s   = k^T q                       (keys j on partitions, queries i free)
  e   = exp(s * C^-0.5)             (logits are O(1); no max-subtraction needed)
  S   = ones^T e                    (softmax denominators via PE reduction)
  o   = vT^T e * (1/S)              (1/S broadcast across partitions via DRAM bounce)
  y   = x + proj_w @ o + proj_b
All matmuls run as float32r (fp32 storage, single-pass PE mode). The two
samples' phases are interleaved in emission order so the second sample's
GroupNorm (DVE) hides under the first sample's attention matmuls (PE).
"""

import numpy as np

import concourse.bass as bass
import concourse.tile as tile
from concourse import bacc, mybir
from concourse.bass_utils import run_bass_kernel_spmd

B, C, H, W = 16, 512, 32, 32
N = H * W              # 1024 spatial positions
G = 32                 # groups
GS = C // G            # 16 channels per group
NCORES = 8
SPC = B // NCORES      # samples per core
EPS = 1e-6
SCALE = float(C) ** -0.5
KT = C // 128          # 4 channel tiles of 128
NT = N // 128          # 8 spatial tiles of 128
NH = N // 512          # 2 free-dim halves of 512

F32 = mybir.dt.float32
F32R = mybir.dt.float32r

_BUILD_CACHE = {}
LAST_RESULT = None  # BassKernelResults of the most recent run (for test harness)


def _build():
    nc = bacc.Bacc("TRN2", target_bir_lowering=False, debug=False)

    x_ext = nc.declare_dram_parameter("x", [SPC, C, N], F32, isOutput=False)
    qkvwT_ext = nc.declare_dram_parameter("qkvwT", [C, 3 * C], F32R, isOutput=False)
    projwT_ext = nc.declare_dram_parameter("projwT", [C, C], F32R, isOutput=False)
    qkvbc_ext = nc.declare_dram_parameter("qkvb_col", [128, 12], F32, isOutput=False)
    cst_ext = nc.declare_dram_parameter("consts12", [128, 12], F32, isOutput=False)
    vbbc_ext = nc.declare_dram_parameter("vb_bc", [128, C], F32, isOutput=False)
    ind_ext = nc.declare_dram_parameter("ind16", [128, 8], F32R, isOutput=False)
    indT_ext = nc.declare_dram_parameter("ind16T", [8, 128], F32R, isOutput=False)
    ones_ext = nc.declare_dram_parameter("ones", [128], F32R, isOutput=False)
    y_ext = nc.declare_dram_parameter("y", [SPC, C, N], F32, isOutput=True)

    sdram = nc.dram_tensor("rs_bounce", [SPC, N], F32)

    Identity = mybir.ActivationFunctionType.Identity
    Exp = mybir.ActivationFunctionType.Exp
    Sqrt = mybir.ActivationFunctionType.Sqrt
    Square = mybir.ActivationFunctionType.Square
    mult = mybir.AluOpType.mult
    add = mybir.AluOpType.add

    with tile.TileContext(nc) as tc:
        with (
            tc.tile_pool(name="wpool", bufs=1) as wpool,
            tc.tile_pool(name="cpool", bufs=1) as cpool,
            tc.tile_pool(name="xpool", bufs=2) as xpool,
            tc.tile_pool(name="hpool", bufs=1) as hpool,
            tc.tile_pool(name="qpool", bufs=1) as qpool,
            tc.tile_pool(name="kpool", bufs=1) as kpool,
            tc.tile_pool(name="vpool", bufs=1) as vpool,
            tc.tile_pool(name="epool", bufs=1) as epool,
            tc.tile_pool(name="opool", bufs=1) as opool,
            tc.tile_pool(name="gnpool", bufs=2) as gnpool,
            tc.tile_pool(name="spool", bufs=1) as spool,
            tc.tile_pool(name="ps", bufs=8, space="PSUM") as ps,
        ):
            # ---- x sample 0 first (256KB chunks across queues) ----
            x_tiles = []
            for s in range(SPC):
                x_tiles.append(
                    xpool.tile([128, KT, N], F32, tag="x", name=f"x_sb{s}")
                )
            for kt in range(KT):
                for sg in range(2):
                    nc.sync.dma_start(
                        out=x_tiles[0][:, kt, sg * 512 : (sg + 1) * 512],
                        in_=x_ext.ap()[
                            0, kt * 128 : (kt + 1) * 128, sg * 512 : (sg + 1) * 512
                        ],
                    )

            # ---- small constants (host-pre-shaped, contiguous loads) ----
            qkvb_col = cpool.tile([128, 12], F32)
            nc.sync.dma_start(out=qkvb_col, in_=qkvbc_ext.ap())
            cst_sb = cpool.tile([128, 12], F32)
            nc.sync.dma_start(out=cst_sb, in_=cst_ext.ap())
            nw_sb = cst_sb[:, 0:4]
            nb_sb = cst_sb[:, 4:8]
            pb_col = cst_sb[:, 8:12]
            vb_bc = cpool.tile([128, C], F32)
            nc.sync.dma_start(out=vb_bc, in_=vbbc_ext.ap())
            ind_sb = cpool.tile([128, 8], F32R)
            nc.sync.dma_start(out=ind_sb, in_=ind_ext.ap())
            indT_sb = cpool.tile([8, 128], F32R)
            nc.sync.dma_start(out=indT_sb, in_=indT_ext.ap())
            ones_col = cpool.tile([128, 1], F32R)
            nc.sync.dma_start(out=ones_col, in_=ones_ext.ap().unsqueeze(1))
            eps_sb = cpool.tile([8, 1], F32)
            nc.vector.memset(eps_sb, EPS)
            warm_sb = cpool.tile([8, 1], F32)
            nc.scalar.activation(out=warm_sb, in_=eps_sb, func=Identity)
            nc.scalar.activation(out=warm_sb, in_=eps_sb, func=Sqrt)
            nc.scalar.activation(out=warm_sb, in_=eps_sb, func=Exp)

            # ---- weights (v columns first per kt), then x1, then proj ----
            qkvw_sb = wpool.tile([128, KT, 3 * C], F32R)
            projw_sb = wpool.tile([128, KT, C], F32R)
            for kt in range(KT):
                for chunk in (2, 0, 1):  # v, q, k column blocks
                    nc.sync.dma_start(
                        out=qkvw_sb[:, kt, chunk * C : (chunk + 1) * C],
                        in_=qkvwT_ext.ap()[
                            kt * 128 : (kt + 1) * 128, chunk * C : (chunk + 1) * C
                        ],
                    )
            for kt in range(KT):
                for sg in range(2):
                    nc.sync.dma_start(
                        out=x_tiles[1][:, kt, sg * 512 : (sg + 1) * 512],
                        in_=x_ext.ap()[
                            1, kt * 128 : (kt + 1) * 128, sg * 512 : (sg + 1) * 512
                        ],
                    )
            for kt in range(KT):
                nc.sync.dma_start(
                    out=projw_sb[:, kt, :],
                    in_=projwT_ext.ap()[kt * 128 : (kt + 1) * 128, :],
                )

            def gn_stats(s, act_split):
                """GroupNorm stats for sample s -> mr (8,KT,2) F32R [mean,rstd].

                act_split: compute kt2/kt3 moments on ScalarE (Identity/Square
                with accum_out) so the DVE and ACT halves run concurrently —
                used for sample 0 where head latency is PE-gating.
                """
                x_sb = x_tiles[s]
                ps_gs = ps.tile([8, KT, 2], F32, tag="mm", name=f"ps_gs{s}")
                dve_kts = (0, 1) if act_split else (0, 1, 2, 3)
                for kt in dve_kts:
                    stats = gnpool.tile(
                        [128, 2, 6], F32, tag=f"stats{kt}", name=f"stats{s}_{kt}"
                    )
                    for sg in range(2):
                        nc.vector.bn_stats(
                            out=stats[:, sg, :],
                            in_=x_sb[:, kt, sg * 512 : (sg + 1) * 512],
                        )
                    mv = gnpool.tile([128, 2], F32, tag=f"mv{kt}", name=f"mv{s}_{kt}")
                    nc.vector.bn_aggr(out=mv, in_=stats)
                    s2f = gnpool.tile(
                        [128, 2], F32, tag=f"s2f{kt}", name=f"s2f{s}_{kt}"
                    )
                    # [E[x], E[x^2]] = [mean, var + mean^2]
                    nc.vector.tensor_mul(s2f[:, 1:2], mv[:, 0:1], mv[:, 0:1])
                    nc.vector.tensor_add(s2f[:, 1:2], s2f[:, 1:2], mv[:, 1:2])
                    nc.vector.tensor_copy(s2f[:, 0:1], mv[:, 0:1])
                    s2 = gnpool.tile(
                        [128, 2], F32R, tag=f"s2_{kt}", name=f"s2_{s}_{kt}"
                    )
                    nc.vector.tensor_copy(s2, s2f)
                    nc.tensor.matmul(
                        ps_gs[:, kt, :], ind_sb, s2, start=True, stop=True
                    )
                if act_split:
                    for kt in (2, 3):
                        s2f = gnpool.tile(
                            [128, 2], F32, tag=f"s2f{kt}", name=f"s2f{s}_{kt}"
                        )
                        scr = gnpool.tile(
                            [128, N], F32, tag="gnscr", name=f"scr{s}_{kt}"
                        )
                        # E[x] and E[x^2] via ScalarE accumulators (exact pwp)
                        nc.scalar.activation(
                            out=scr, in_=x_sb[:, kt, :], func=Identity,
                            scale=1.0 / N, accum_out=s2f[:, 0:1],
                        )
                        scr2 = gnpool.tile(
                            [128, N], F32, tag="gnscr", name=f"scr2{s}_{kt}"
                        )
                        nc.scalar.activation(
                            out=scr2, in_=x_sb[:, kt, :], func=Square,
                            scale=N ** -0.5, accum_out=s2f[:, 1:2],
                        )
                        s2 = gnpool.tile(
                            [128, 2], F32R, tag=f"s2_{kt}", name=f"s2_{s}_{kt}"
                        )
                        nc.vector.tensor_copy(s2, s2f)
                        nc.tensor.matmul(
                            ps_gs[:, kt, :], ind_sb, s2, start=True, stop=True
                        )
                # group mean / E[x^2] (divide by 16 partitions per group)
                gs = gnpool.tile([8, KT, 2], F32, tag="gs", name=f"gs{s}")
                nc.vector.tensor_scalar_mul(gs, ps_gs, 1.0 / GS)
                msq = gnpool.tile([8, KT], F32, tag="msq", name=f"msq{s}")
                nc.vector.tensor_mul(msq, gs[:, :, 0], gs[:, :, 0])
                nc.vector.tensor_sub(gs[:, :, 1], gs[:, :, 1], msq)
                nc.scalar.activation(
                    out=gs[:, :, 1], in_=gs[:, :, 1], func=Sqrt, bias=eps_sb
                )
                nc.vector.reciprocal(gs[:, :, 1], gs[:, :, 1])
                # rounded copy [mean, rstd] feeding the broadcast matmul
                mr = gnpool.tile([8, KT, 2], F32R, tag="mr", name=f"mr{s}")
                nc.vector.tensor_copy(mr, gs)
                return mr

            def gn_apply(s, mr, engines):
                """Broadcast stats to channels and apply x*scale+bias -> h."""
                x_sb = x_tiles[s]
                h_sb = hpool.tile([128, KT, N], F32R, tag="h", name=f"h{s}")
                for kt in range(KT):
                    ps_bc = ps.tile([128, 2], F32, tag="mm", name=f"ps_bc{s}_{kt}")
                    nc.tensor.matmul(
                        ps_bc, indT_sb, mr[:, kt, :], start=True, stop=True
                    )
                    scb = gnpool.tile(
                        [128, 2], F32, tag=f"scb{kt}", name=f"scb{s}_{kt}"
                    )
                    nc.vector.tensor_mul(
                        scb[:, 0:1], ps_bc[:, 1:2], nw_sb[:, kt : kt + 1]
                    )
                    nc.vector.tensor_mul(scb[:, 1:2], ps_bc[:, 0:1], scb[:, 0:1])
                    nc.vector.tensor_sub(
                        scb[:, 1:2], nb_sb[:, kt : kt + 1], scb[:, 1:2]
                    )
                    if engines[kt] == "v":
                        nc.vector.tensor_scalar(
                            out=h_sb[:, kt, :],
                            in0=x_sb[:, kt, :],
                            scalar1=scb[:, 0:1],
                            scalar2=scb[:, 1:2],
                            op0=mult,
                            op1=add,
                        )
                    else:
                        nc.scalar.activation(
                            out=h_sb[:, kt, :], in_=x_sb[:, kt, :],
                            func=Identity, scale=scb[:, 0:1], bias=scb[:, 1:2],
                        )
                return h_sb

            def v_transposed(h_sb):
                """vT = h^T @ v_w^T (+ v bias broadcast along free dim)."""
                vT_sb = vpool.tile([128, NT, C], F32R, tag="vT")
                for nt in range(NT):
                    pm = ps.tile([128, 512], F32, tag="mm")
                    for kt in range(KT):
                        nc.tensor.matmul(
                            pm,
                            h_sb[:, kt, nt * 128 : (nt + 1) * 128],
                            qkvw_sb[:, kt, 2 * C : 3 * C],
                            start=(kt == 0),
                            stop=(kt == KT - 1),
                        )
                    nc.vector.tensor_add(vT_sb[:, nt, :], pm, vb_bc)
                return vT_sb

            def qk(h_sb):
                q_sb = qpool.tile([128, KT, N], F32R, tag="q")
                k_sb = kpool.tile([128, KT, N], F32R, tag="k")
                for ih in range(NH):
                    for ot in range(8):
                        dest = q_sb if ot < 4 else k_sb
                        oc = ot % 4
                        pm = ps.tile([128, 512], F32, tag="mm")
                        for kt in range(KT):
                            nc.tensor.matmul(
                                pm,
                                qkvw_sb[:, kt, ot * 128 : (ot + 1) * 128],
                                h_sb[:, kt, ih * 512 : (ih + 1) * 512],
                                start=(kt == 0),
                                stop=(kt == KT - 1),
                            )
                        nc.scalar.activation(
                            out=dest[:, oc, ih * 512 : (ih + 1) * 512],
                            in_=pm,
                            func=Identity,
                            bias=qkvb_col[:, ot : ot + 1],
                        )
                return q_sb, k_sb

            def attn_scores(s, q_sb, k_sb):
                # s = k^T q (keys on partitions); e = exp(s * scale)
                e_sb = epool.tile([128, NT, N], F32R, tag="e", name=f"e{s}")
                for jt in range(NT):
                    for ih in range(NH):
                        pm = ps.tile([128, 512], F32, tag="mm")
                        for ot in range(KT):
                            nc.tensor.matmul(
                                pm,
                                k_sb[:, ot, jt * 128 : (jt + 1) * 128],
                                q_sb[:, ot, ih * 512 : (ih + 1) * 512],
                                start=(ot == 0),
                                stop=(ot == KT - 1),
                            )
                        nc.scalar.activation(
                            out=e_sb[:, jt, ih * 512 : (ih + 1) * 512],
                            in_=pm,
                            func=Exp,
                            scale=SCALE,
                        )
                return e_sb

            def attn_out(s, e_sb, vT_sb):
                # softmax denominators S = sum_j e; 1/S broadcast via DRAM
                recipS = spool.tile([1, N], F32, tag="recipS", name=f"recipS{s}")
                for ih in range(NH):
                    pS = ps.tile([1, 512], F32, tag="mm")
                    for jt in range(NT):
                        nc.tensor.matmul(
                            pS,
                            ones_col,
                            e_sb[:, jt, ih * 512 : (ih + 1) * 512],
                            start=(jt == 0),
                            stop=(jt == NT - 1),
                        )
                    nc.vector.reciprocal_approx_fast(
                        out=recipS[:, ih * 512 : (ih + 1) * 512], in_=pS
                    )
                nc.sync.dma_start(out=sdram.ap()[s].unsqueeze(0), in_=recipS)
                rSbc = spool.tile([128, N], F32, tag="rSbc", name=f"rSbc{s}")
                nc.sync.dma_start(
                    out=rSbc, in_=sdram.ap()[s].partition_broadcast(128)
                )
                # o = vT^T @ e, normalized by 1/S
                o_sb = opool.tile([128, KT, N], F32R, tag="o", name=f"o{s}")
                for ct in range(KT):
                    for ih in range(NH):
                        pm = ps.tile([128, 512], F32, tag="mm")
                        for jt in range(NT):
                            nc.tensor.matmul(
                                pm,
                                vT_sb[:, jt, ct * 128 : (ct + 1) * 128],
                                e_sb[:, jt, ih * 512 : (ih + 1) * 512],
                                start=(jt == 0),
                                stop=(jt == NT - 1),
                            )
                        nc.vector.tensor_mul(
                            o_sb[:, ct, ih * 512 : (ih + 1) * 512],
                            pm,
                            rSbc[:, ih * 512 : (ih + 1) * 512],
                        )
                return o_sb

            def proj_resid(s, o_sb, x_sb):
                # residual accumulates in place into the (now dead) x tile
                for ct2 in range(KT):
                    for ih in range(NH):
                        pm = ps.tile([128, 512], F32, tag="mm")
                        for ckt in range(KT):
                            nc.tensor.matmul(
                                pm,
                                projw_sb[:, ckt, ct2 * 128 : (ct2 + 1) * 128],
                                o_sb[:, ckt, ih * 512 : (ih + 1) * 512],
                                start=(ckt == 0),
                                stop=(ckt == KT - 1),
                            )
                        # + proj bias, in place on PSUM (ScalarE)
                        nc.scalar.activation(
                            out=pm, in_=pm, func=Identity,
                            bias=pb_col[:, ct2 : ct2 + 1],
                        )
                        # + residual, in place into x
                        nc.vector.tensor_add(
                            x_sb[:, ct2, ih * 512 : (ih + 1) * 512],
                            pm,
                            x_sb[:, ct2, ih * 512 : (ih + 1) * 512],
                        )
                        nc.gpsimd.dma_start(
                            out=y_ext.ap()[
                                s,
                                ct2 * 128 : (ct2 + 1) * 128,
                                ih * 512 : (ih + 1) * 512,
                            ],
                            in_=x_sb[:, ct2, ih * 512 : (ih + 1) * 512],
                        )

            # ---- interleaved two-sample schedule ----
            mr0 = gn_stats(0, act_split=False)
            h0 = gn_apply(0, mr0, engines="avav")
            vT0 = v_transposed(h0)
            q0, k0 = qk(h0)
            mr1 = gn_stats(1, act_split=False)  # DVE-only, hides under s0 attn
            e0 = attn_scores(0, q0, k0)
            h1 = gn_apply(1, mr1, engines="aaaa")  # ACT applies, DVE stays free
            o0 = attn_out(0, e0, vT0)
            proj_resid(0, o0, x_tiles[0])
            vT1 = v_transposed(h1)
            q1, k1 = qk(h1)
            e1 = attn_scores(1, q1, k1)
            o1 = attn_out(1, e1, vT1)
            proj_resid(1, o1, x_tiles[1])

    nc.compile()
    return nc


def _get_nc():
    if "nc" not in _BUILD_CACHE:
        _BUILD_CACHE["nc"] = _build()
    return _BUILD_CACHE["nc"]


def kernel(x, norm_w, norm_b, qkv_w, qkv_b, proj_w, proj_b, _trace=False):
    global LAST_RESULT
    nc = _get_nc()

    x = np.asarray(x, dtype=np.float32).reshape(B, C, N)
    qkvwT = np.ascontiguousarray(np.asarray(qkv_w, dtype=np.float32).T)
    projwT = np.ascontiguousarray(np.asarray(proj_w, dtype=np.float32).T)
    ind16 = np.zeros((128, 8), dtype=np.float32)
    for p in range(128):
        ind16[p, p // GS] = 1.0
    ind16T = np.ascontiguousarray(ind16.T)

    norm_w = np.asarray(norm_w, dtype=np.float32)
    norm_b = np.asarray(norm_b, dtype=np.float32)
    qkv_b = np.asarray(qkv_b, dtype=np.float32)
    proj_b = np.asarray(proj_b, dtype=np.float32)
    # per-o-tile bias columns: col t holds bias[t*128 : (t+1)*128]
    qkvb_col = np.ascontiguousarray(qkv_b.reshape(12, 128).T)
    consts12 = np.ascontiguousarray(
        np.concatenate(
            [
                norm_w.reshape(KT, 128).T,
                norm_b.reshape(KT, 128).T,
                proj_b.reshape(KT, 128).T,
            ],
            axis=1,
        )
    )
    vb_bc = np.ascontiguousarray(
        np.broadcast_to(qkv_b[2 * C : 3 * C], (128, C))
    )
    shared = {
        "qkvwT": qkvwT,
        "projwT": projwT,
        "qkvb_col": qkvb_col,
        "consts12": consts12,
        "vb_bc": vb_bc,
        "ind16": ind16,
        "ind16T": ind16T,
        "ones": np.ones(128, dtype=np.float32),
    }
    in_maps = [
        {"x": np.ascontiguousarray(x[c * SPC : (c + 1) * SPC]), **shared}
        for c in range(NCORES)
    ]
    res = run_bass_kernel_spmd(nc, in_maps, list(range(NCORES)), trace=_trace)
    LAST_RESULT = res
    out = np.concatenate([res.results[i]["y"] for i in range(NCORES)], axis=0)
    return out.reshape(B, C, H, W)
